# revision 6
# baseline (speedup 1.0000x reference)
"""DeepFM Trainium2 kernel (8-core batch-data-parallel).

Model (from the reference):
  emb[b,f,:]  = emb_tables[f, sparse[b,f], :]          # [B, 26, 16]
  lin[b,f]    = lin_tables[f, sparse[b,f]]
  out[b] = gbias + sum_f lin[b,f] + dense[b]@w_ld + lin_dense_b
         + 0.5*sum_e((sum_f emb)^2 - sum_f emb^2)
         + MLP(bn0(concat(emb.flat, dense)))           # 429 -> 128 -> 64 -> 1

Strategy:
  * Shard the batch 8 ways (2048 samples/core); replicate tables in HBM
    (table residency is free at exec time - only gathered rows move).
  * Host prepends lin to emb rows -> combined table [26*100001, 17] so one
    indirect-DMA descriptor fetches emb row + lin scalar (68 B).
  * Gather is batch-major: G[128 samples, 26*17]. FM + linear terms are
    computed batch-major with DVE strided reduces.
  * The MLP needs feature-major layout: PE transposes G in 128-col chunks,
    then accumulates W1 chunk matmuls in PSUM. BN0/BN1/BN2 (eval mode) are
    affine -> folded into W1/W2 and biases on the host. Dense features are
    shipped pre-transposed and join via a K=13 matmul accumulation.
  * Batch-major partial (FM + linear + const) is PE-transposed [128,1]->[1,128]
    and added to the MLP row; one contiguous DMA per core writes the output.
"""

import copy
import json

import numpy as np

import concourse.bass as bass
import concourse.mybir as mybir
import concourse.tile as tile
from concourse.bass_utils import run_bass_kernel_spmd


NUM_SWDGE_QUEUES = 4


def _install_wait_split_patch():
    """Compat shim: this walrus build rejects instructions carrying more than
    one sync-wait ("Too many sync wait commands" in setupSyncWait). Post-pass
    the serialized BIR: hoist all-but-the-last wait of any multi-wait
    instruction onto standalone EventSemaphore instructions inserted just
    before it on the same engine — semantically identical, since each engine
    sequencer processes its stream in order.

    Also round-robins indirect-DMA (qPoolDynamic) instructions across the
    declared SWDGE queues so descriptor generation can spread over Q7
    contexts."""
    if getattr(bass.Bass, "_wait_split_patched", False):
        return
    orig = bass.Bass.to_json_bytes

    def to_json_bytes(self):
        m = json.loads(orig(self))
        qi = 0
        for fn in m.get("functions", []):
            for blk in fn.get("blocks", []):
                for inst in blk.get("instructions", []):
                    if inst.get("queue") == "qPoolDynamic" and NUM_SWDGE_QUEUES > 1:
                        sfx = qi % NUM_SWDGE_QUEUES
                        inst["queue"] = f"qPoolDynamic{sfx or ''}"
                        qi += 1
        for fn in m.get("functions", []):
            # find a template EventSemaphore so synthetic insts carry every
            # field this BIR schema expects
            template = None
            for blk in fn.get("blocks", []):
                for inst in blk.get("instructions", []):
                    if inst.get("opcode") == "EventSemaphore":
                        template = inst
                        break
                if template:
                    break
            for blk in fn.get("blocks", []):
                insts = blk.get("instructions")
                if not insts:
                    continue
                out = []
                for inst in insts:
                    si = inst.get("sync_info")
                    waits = (si or {}).get("on_wait") or []
                    if len(waits) > 1:
                        for j, extra in enumerate(waits[:-1]):
                            if template is not None:
                                ev = copy.deepcopy(template)
                            else:
                                ev = {"opcode": "EventSemaphore", "ins": [], "outs": []}
                            ev["name"] = f"{inst['name']}_wsplit{j}"
                            ev["engine"] = inst["engine"]
                            ev["sync_info"] = {"on_update": [], "on_wait": [extra]}
                            out.append(ev)
                        si["on_wait"] = [waits[-1]]
                    out.append(inst)
                blk["instructions"] = out
        return json.dumps(m).encode()

    bass.Bass.to_json_bytes = to_json_bytes
    bass.Bass._wait_split_patched = True


_install_wait_split_patch()

# Problem constants (hardcoded per harness contract)
B, F, D, E, V1 = 16384, 26, 13, 16, 100001
H1, H2 = 128, 64
EPS = 1e-5
NCORES = 8
BC = B // NCORES            # 2048 samples per core
P = 128                     # partitions / batch tile
T = BC // P                 # 16 batch tiles per core
R = E + 1                   # 17 floats per combined table row
CE = F * R                  # 442 gathered cols per sample
GROUP = 4                   # batch tiles per indirect-DMA instruction
CHUNKS = [128, 128, 128, CE - 384]   # transpose/matmul K-chunk widths (..., 58)
f32 = mybir.dt.float32
i32 = mybir.dt.int32

FuncT = mybir.ActivationFunctionType
Alu = mybir.AluOpType
Ax = mybir.AxisListType


def build_bass(const_total: float) -> bass.Bass:
    nc = bass.Bass(num_swdge_queues=NUM_SWDGE_QUEUES)

    comb = nc.dram_tensor("comb", [F * V1, R], f32, kind="ExternalInput")
    gidx = nc.dram_tensor("gidx", [P, T * F], i32, kind="ExternalInput")
    denseT = nc.dram_tensor("denseT", [D, BC], f32, kind="ExternalInput")
    w1t = nc.dram_tensor("w1t", [P, 4 * P], f32, kind="ExternalInput")
    wd1 = nc.dram_tensor("wd1", [D, H1], f32, kind="ExternalInput")
    w2t = nc.dram_tensor("w2t", [H1, H2], f32, kind="ExternalInput")
    wot = nc.dram_tensor("wot", [H2, 1], f32, kind="ExternalInput")
    wdl = nc.dram_tensor("wdl", [D, 1], f32, kind="ExternalInput")
    b1e = nc.dram_tensor("b1e", [H1, 1], f32, kind="ExternalInput")
    b2e = nc.dram_tensor("b2e", [H2, 1], f32, kind="ExternalInput")
    ident = nc.dram_tensor("ident", [P, P], f32, kind="ExternalInput")
    out = nc.dram_tensor("out", [1, BC], f32, kind="ExternalOutput")

    with tile.TileContext(nc) as tc:
        with (
            tc.tile_pool(name="const", bufs=1) as cpool,
            tc.tile_pool(name="gath", bufs=4) as gpool,
            tc.tile_pool(name="xt", bufs=2) as xpool,
            tc.tile_pool(name="small", bufs=3) as spool,
            tc.tile_pool(name="ps_xt", bufs=2, space="PSUM") as ppool,
            tc.tile_pool(name="ps_h1", bufs=2, space="PSUM") as hpool,
            tc.tile_pool(name="ps_sm", bufs=1, space="PSUM") as qpool,
        ):
            # ---- preloads (once) ----
            gidx_sb = cpool.tile([P, T * F], i32)
            nc.sync.dma_start(out=gidx_sb[:], in_=gidx[:])
            denseT_sb = cpool.tile([D, BC], f32)
            nc.sync.dma_start(out=denseT_sb[:], in_=denseT[:])
            w1t_sb = cpool.tile([P, 4 * P], f32)
            nc.sync.dma_start(out=w1t_sb[:], in_=w1t[:])
            wd1_sb = cpool.tile([D, H1], f32)
            nc.sync.dma_start(out=wd1_sb[:], in_=wd1[:])
            w2t_sb = cpool.tile([H1, H2], f32)
            nc.sync.dma_start(out=w2t_sb[:], in_=w2t[:])
            wot_sb = cpool.tile([H2, 1], f32)
            nc.sync.dma_start(out=wot_sb[:], in_=wot[:])
            wdl_sb = cpool.tile([D, 1], f32)
            nc.sync.dma_start(out=wdl_sb[:], in_=wdl[:])
            b1e_sb = cpool.tile([H1, 1], f32)
            nc.sync.dma_start(out=b1e_sb[:], in_=b1e[:])
            b2e_sb = cpool.tile([H2, 1], f32)
            nc.sync.dma_start(out=b2e_sb[:], in_=b2e[:])
            ident_sb = cpool.tile([P, P], f32)
            nc.sync.dma_start(out=ident_sb[:], in_=ident[:])
            out_sb = cpool.tile([1, BC], f32)

            for t in range(T):
                # HW indirect DMA consumes ONE offset per dest partition row
                # (multi-offset APs silently misbehave - probed), so each
                # instruction gathers 128 rows (one feature, one batch tile).
                # The 26 instructions per tile round-robin across SWDGE queues
                # via the to_json_bytes post-pass.
                Gt_tile = gpool.tile([P, CE], f32, tag="Gt")
                for f in range(F):
                    nc.gpsimd.indirect_dma_start(
                        out=Gt_tile[:, f * R : (f + 1) * R],
                        out_offset=None,
                        in_=comb[:],
                        in_offset=bass.IndirectOffsetOnAxis(
                            ap=gidx_sb[:, t * F + f : t * F + f + 1],
                            axis=0,
                        ),
                    )
                if True:
                    Gt = Gt_tile[:]

                    # ---- batch-major FM / linear-sparse ----
                    # s_bm[:, e<16] = sum_f emb[b,f,e]; s_bm[:, 16] = sum_f lin
                    s_bm = spool.tile([P, R], f32, tag="s_bm")
                    nc.vector.reduce_sum(
                        out=s_bm[:],
                        in_=Gt.rearrange("p (f e) -> p e f", e=R),
                        axis=Ax.X,
                    )
                    # sqs = sum(emb^2)  (ACT Square with free-dim accumulate)
                    dump = spool.tile([P, F * E], f32, tag="dump")
                    sqs = spool.tile([P, 1], f32, tag="sqs")
                    embv = Gt.rearrange("p (f e) -> p f e", e=R)[:, :, 0:E]
                    nc.scalar.activation(
                        out=dump[:].rearrange("p (f e) -> p f e", e=E),
                        in_=embv,
                        func=FuncT.Square,
                        accum_out=sqs[:],
                    )
                    # ssum = sum(s^2)
                    dump2 = spool.tile([P, E], f32, tag="dump2")
                    ssum = spool.tile([P, 1], f32, tag="ssum")
                    nc.scalar.activation(
                        out=dump2[:],
                        in_=s_bm[:, 0:E],
                        func=FuncT.Square,
                        accum_out=ssum[:],
                    )
                    # total_b = 0.5*(ssum - sqs) + const + lin_sparse
                    tmp = spool.tile([P, 1], f32, tag="tmp")
                    nc.vector.tensor_sub(out=tmp[:], in0=ssum[:], in1=sqs[:])
                    tmp2 = spool.tile([P, 1], f32, tag="tmp2")
                    nc.vector.tensor_scalar(
                        out=tmp2[:],
                        in0=tmp[:],
                        scalar1=0.5,
                        scalar2=float(const_total),
                        op0=Alu.mult,
                        op1=Alu.add,
                    )
                    totb = spool.tile([P, 1], f32, tag="totb")
                    nc.vector.tensor_add(out=totb[:], in0=tmp2[:], in1=s_bm[:, E:R])

                    # ---- transpose to feature-major ----
                    xTp = ppool.tile([P, 4 * P], f32, tag="xTp")
                    for k in range(4):
                        kk = CHUNKS[k]
                        nc.tensor.transpose(
                            out=xTp[0:kk, k * P : (k + 1) * P],
                            in_=Gt[:, k * P : k * P + kk],
                            identity=ident_sb[:],
                        )
                    xT = xpool.tile([P, 4 * P], f32, tag="xT")
                    nc.vector.tensor_copy(out=xT[:, 0 : 2 * P], in_=xTp[:, 0 : 2 * P])
                    nc.scalar.copy(out=xT[:, 2 * P : 3 * P], in_=xTp[:, 2 * P : 3 * P])
                    nc.scalar.copy(
                        out=xT[0 : CHUNKS[3], 3 * P : 4 * P],
                        in_=xTp[0 : CHUNKS[3], 3 * P : 4 * P],
                    )

                    # ---- MLP (BN folded into weights) ----
                    h1p = hpool.tile([P, P], f32, tag="h1p")
                    for k in range(4):
                        kk = CHUNKS[k]
                        nc.tensor.matmul(
                            out=h1p[:],
                            lhsT=w1t_sb[0:kk, k * P : (k + 1) * P],
                            rhs=xT[0:kk, k * P : (k + 1) * P],
                            start=(k == 0),
                            stop=False,
                        )
                    nc.tensor.matmul(
                        out=h1p[:],
                        lhsT=wd1_sb[:],
                        rhs=denseT_sb[:, t * P : (t + 1) * P],
                        start=False,
                        stop=True,
                    )
                    h1s = xpool.tile([P, P], f32, tag="h1s")
                    nc.scalar.activation(
                        out=h1s[:], in_=h1p[:], func=FuncT.Relu, bias=b1e_sb[:, 0:1]
                    )
                    h2p = qpool.tile([H2, P], f32, tag="h2p", bufs=2)
                    nc.tensor.matmul(
                        out=h2p[:], lhsT=w2t_sb[:], rhs=h1s[:], start=True, stop=True
                    )
                    h2s = spool.tile([H2, P], f32, tag="h2s")
                    nc.scalar.activation(
                        out=h2s[:], in_=h2p[:], func=FuncT.Relu, bias=b2e_sb[:, 0:1]
                    )
                    # deep_out + linear_dense, accumulated in one PSUM row
                    dop = qpool.tile([1, P], f32, tag="dop")
                    nc.tensor.matmul(
                        out=dop[:], lhsT=wot_sb[:], rhs=h2s[:], start=True, stop=False
                    )
                    nc.tensor.matmul(
                        out=dop[:],
                        lhsT=wdl_sb[:],
                        rhs=denseT_sb[:, t * P : (t + 1) * P],
                        start=False,
                        stop=False,
                    )
                    # merge batch-major partial: totb.T @ I accumulated in PSUM
                    nc.tensor.matmul(
                        out=dop[:],
                        lhsT=totb[:, 0:1],
                        rhs=ident_sb[:],
                        start=False,
                        stop=True,
                    )
                    nc.vector.tensor_copy(
                        out=out_sb[0:1, t * P : (t + 1) * P], in_=dop[:]
                    )

            nc.sync.dma_start(out=out[:], in_=out_sb[:])

    return nc


def _prep_host(inputs):
    """Host-side input preprocessing: combined table, global indices, folded
    BN/MLP weights. Returns (shared_map, per_core_maps, const_total)."""
    f = np.float32
    sparse = np.asarray(inputs["sparse_inputs"])
    dense = np.asarray(inputs["dense_inputs"], f)
    embt = np.asarray(inputs["emb_tables"], f)
    lint = np.asarray(inputs["lin_tables"], f)

    comb = np.concatenate([embt, lint[:, :, None]], axis=2).reshape(F * V1, R)
    comb = np.ascontiguousarray(comb)

    gidx_g = (sparse.astype(np.int64) + (np.arange(F, dtype=np.int64) * V1)[None, :])
    gidx_g = gidx_g.astype(np.int32)

    # BN folds (eval-mode affine)
    a0 = (inputs["bn0_g"] / np.sqrt(inputs["bn0_v"] + EPS)).astype(f)
    c0 = (inputs["bn0_b"] - inputs["bn0_m"] * a0).astype(f)
    a1 = (inputs["bn1_g"] / np.sqrt(inputs["bn1_v"] + EPS)).astype(f)
    c1 = (inputs["bn1_b"] - inputs["bn1_m"] * a1).astype(f)
    a2 = (inputs["bn2_g"] / np.sqrt(inputs["bn2_v"] + EPS)).astype(f)
    c2 = (inputs["bn2_b"] - inputs["bn2_m"] * a2).astype(f)

    W1 = np.asarray(inputs["W1"], f)
    b1 = np.asarray(inputs["b1"], f)
    W2 = np.asarray(inputs["W2"], f)
    b2 = np.asarray(inputs["b2"], f)
    Wo = np.asarray(inputs["Wo"], f)

    W1eff = (a1[:, None] * W1) * a0[None, :]          # [128, 429]
    b1eff = (a1 * (W1 @ c0 + b1) + c1).astype(f)      # [128]
    W2eff = a2[:, None] * W2                          # [64, 128]
    b2eff = (a2 * b2 + c2).astype(f)                  # [64]

    # W1 emb cols -> 17-stride position space, chunked+transposed for lhsT
    w1p = np.zeros((H1, 4 * P), f)
    pos = (np.arange(F)[:, None] * R + np.arange(E)[None, :]).ravel()
    w1p[:, pos] = W1eff[:, : F * E]
    w1t_dev = np.ascontiguousarray(
        w1p.T.reshape(4, P, H1).transpose(1, 0, 2).reshape(P, 4 * H1)
    )
    wd1 = np.ascontiguousarray(W1eff[:, F * E :].T)   # [13, 128]
    w2t = np.ascontiguousarray(W2eff.T)               # [128, 64]
    wot = np.ascontiguousarray(Wo.reshape(1, H2).T)   # [64, 1]
    wdl = np.ascontiguousarray(
        np.asarray(inputs["lin_dense_w"], f).reshape(D, 1)
    )

    const_total = float(
        np.asarray(inputs["gbias"], f).reshape(-1)[0]
        + np.asarray(inputs["bo"], f).reshape(-1)[0]
        + np.asarray(inputs["lin_dense_b"], f).reshape(-1)[0]
    )

    shared = dict(
        comb=comb,
        w1t=w1t_dev,
        wd1=wd1,
        w2t=w2t,
        wot=wot,
        wdl=wdl,
        b1e=np.ascontiguousarray(b1eff.reshape(H1, 1)),
        b2e=np.ascontiguousarray(b2eff.reshape(H2, 1)),
        ident=np.eye(P, dtype=f),
    )

    per_core = []
    for c in range(NCORES):
        gc = gidx_g[c * BC : (c + 1) * BC]
        gc = np.ascontiguousarray(
            gc.reshape(T, P, F).transpose(1, 0, 2).reshape(P, T * F)
        )
        dc = np.ascontiguousarray(dense[c * BC : (c + 1) * BC].T)  # [13, 2048]
        per_core.append(dict(shared, gidx=gc, denseT=dc))
    return per_core, const_total


def kernel(**inputs) -> np.ndarray:
    per_core, const_total = _prep_host(inputs)
    nc = build_bass(const_total)
    res = run_bass_kernel_spmd(nc, per_core, core_ids=list(range(NCORES)))
    out = np.concatenate(
        [res.results[c]["out"].reshape(-1) for c in range(NCORES)]
    )
    return out.astype(np.float32)

